# revision 1
# baseline (speedup 1.0000x reference)
"""Trainium2 Bass kernel for nn_EvolvableSNN (T=512, B=8, N=4096, LIF SNN).

Strategy
--------
The LIF dynamics with these parameters are sub-threshold: the membrane
potential equilibrium is ~tau_mem*tau_syn*cur ~= 1e-4 * cur, four orders of
magnitude below threshold=1.0, so no neuron ever spikes and the recurrent
feedback term is identically zero.  With zero feedback the scan is a LINEAR
time-invariant filter of the feedforward drive:

    ff    = input[:, :, :512] @ W_in                      # [T, B, N]
    mem_t = DT^2 * sum_{s<=t} g(t-s) * ff_s               # per (b, n)
    g(d)  = (b^(d+1) - a^(d+1)) / (b - a),  a = 1-DT/tau_syn, b = 1-DT/tau_mem
    spikes_t = (mem_t >= threshold)

so mem = GT.T @_time (x @ W_in) -- two chained dense matmuls, fully parallel
across (batch, neuron).  Validity is guarded by a rigorous norm bound
computed on the host:

    max|mem| <= DT^2 * sum_d g(d) * max_row||x_row||_2 * max_col||W_col||_2

(~2e-3 for the target inputs, vs threshold 1.0).  If the bound (inflated by
the mixed-precision error allowance) does not clear min(threshold) by a wide
margin -- or the device-computed max(mem) comes anywhere near threshold --
we fall back to an exact sequential numpy port of the reference.  The first
spike of the no-feedback system coincides with the first spike of the true
system, so "no spikes under linearization" exactly implies correctness.

Device kernel (per core, batch-parallel: core c owns batch c, full N):
  stage 1: xgT[i, t] = sum_s x_c[s, i] * GT[s, t]   (fp8 DoubleRow matmuls,
           GT upper-triangular so the moving range is trimmed)
  stage 2: mem[t, n] = sum_i xgT[i, t] * W[i, n]    (fp8 DoubleRow matmuls,
           512-wide PSUM tiles)
  each PSUM tile is consumed in place (VectorE max-reduce / ScalarE
  Relu-accum, split per IS_MAX); the only output is the [128, 33] tile
  summary (132 bytes per partition) -- no spike map is materialized or
  written to HBM.  The host checks max(mem) < 0.5*threshold (scaled) and
  all Relu sums == 0, then emits the all-zero spike tensor; anything
  unexpected falls back to the exact numpy path.

Numerics: both matmul stages run as fp8-e4m3 DoubleRow (2x PE throughput)
with power-of-two scale factors (sxx on x, sgt on GT, sx/(sxx*sgt) applied
by the PSUM->SBUF copy, sw folded into W on the host); accumulation is
fp32 PSUM throughout, and the tile maxes are exact fp32 reductions of the
PSUM values (= mem * sx * sw).
"""

import math

import numpy as np
import ml_dtypes

import concourse.bass as bass
import concourse.mybir as mybir
import concourse.tile as tile
from concourse import bacc, bass_utils

# Problem constants (hardcoded per harness contract).
T, B, N = 512, 8, 4096
IN = 512          # INPUT_SIZE
DT = 0.001
P = 128           # SBUF partitions
NCORES = 8

# Batch-parallel: core c owns batch c, full neuron range.
NW = N            # neuron columns per core
KI = IN // P      # contraction tiles over input dim (4)
KP = KI // 2      # DoubleRow contraction pair-tiles (2)
KT = T // P       # tiles over time dim (4)
NCH = NW // 512   # 512-wide n chunks per core (8)
F32 = mybir.dt.float32
FP8 = mybir.dt.float8e4
NPFP8 = ml_dtypes.float8_e4m3

MARGIN = 0.1               # abs margin to min(threshold) for the fast path

_compiled = {}             # cached compiled Bass modules
LAST_RES = None            # last device results (for external profiling)

# Which stage-2 PSUM tile goes to the VectorE max-reduce (True) vs the
# ScalarE Relu-accum (False).  Alternating halves the per-engine load; the
# final four tiles are ordered so the slower ScalarE gets the two that
# close EARLIEST (28, 30) and VectorE the later ones, letting both queues
# drain in parallel right after the last matmul.
IS_MAX = tuple(
    (idx % 2 == 0) if idx < 27 else (idx in (27, 29, 31)) for idx in range(33)
)


def _filter_taps(alpha: float, beta: float) -> np.ndarray:
    """g(d) * DT^2 for d = 0..T-1 (float64)."""
    d = np.arange(T, dtype=np.float64)
    if abs(beta - alpha) > 1e-12:
        g = (beta ** (d + 1) - alpha ** (d + 1)) / (beta - alpha)
    else:
        g = (d + 1) * alpha**d
    return g * DT * DT


def _build_gt(alpha: float, beta: float) -> np.ndarray:
    """GT[s, t] = DT^2 * g(t - s) for s <= t else 0 (upper-triangular)."""
    g = _filter_taps(alpha, beta)
    s = np.arange(T)
    diff = s[None, :] - s[:, None]  # diff[s, t] = t - s
    gt = np.where(diff >= 0, g[np.clip(diff, 0, T - 1)], 0.0)
    return gt.astype(np.float32)


def _build_device():
    """Compile the per-core Tile kernel; returns the Bass module.

    Input layouts are pre-packed on the host so every DMA is one large
    fully-contiguous transfer:
      x  [P, KP, 2, IN]        fp8, x[p, kp, i2, i]
                               = x_c[(2kp+i2)*128+p, i] * sxx
      w  [P, NCH, KP, 2, 512]  fp8, w[p, j, kp, i2, n]
                               = W_in[(2kp+i2)*128+p, j*512+n] * sw
      gt [P, KP, 2, T]         fp8, gt[p, kp, i2, t]
                               = GT[(2kp+i2)*128+p, t] * sgt
      sc [P, 2]                col 0: sx/(sxx*sgt) (stage-1 copy scale),
                               col 1: -0.5 * threshold * sx * sw
    Output:
      mx [P, NCH*KT]           f32 per PSUM tile: IS_MAX tiles carry the
                               VectorE max of mem*sx*sw; the rest carry
                               the ScalarE sum of Relu(mem*sx*sw - th/2)
                               (zero iff the whole tile is below th/2)

    Only the two HWDGE rings are used (sync + scalar): each dma_start
    dispatch costs ~0.7us on its sequencer and the SWDGE (gpsimd) ring
    adds its own slow descriptor builds, so critical loads go first on
    separate rings and the W chunks stream behind in consumption order.
    """
    nc = bacc.Bacc(
        "TRN2", target_bir_lowering=False, debug=False, num_devices=NCORES
    )
    x = nc.dram_tensor("x", [P, KP, 2, IN], FP8, kind="ExternalInput").ap()
    w = nc.dram_tensor("w", [P, NCH, KP, 2, 512], FP8, kind="ExternalInput").ap()
    gt = nc.dram_tensor("gt", [P, KP, 2, T], FP8, kind="ExternalInput").ap()
    sc = nc.dram_tensor("sc", [P, 2], F32, kind="ExternalInput").ap()
    mx = nc.dram_tensor("mx", [P, NCH * KT + 1], F32, kind="ExternalOutput").ap()

    with tile.TileContext(nc) as tc:
        with (
            tc.tile_pool(name="const", bufs=1) as cpool,
            tc.tile_pool(name="xin", bufs=1) as xpool,
            tc.tile_pool(name="xg", bufs=1) as xgpool,
            tc.tile_pool(name="junk", bufs=2) as jpool,
            tc.tile_pool(name="ps1", bufs=4, space="PSUM") as ps1,
            tc.tile_pool(name="ps2", bufs=4, space="PSUM") as ps2,
        ):
            # PE p-state warmup: the PE needs ~3.5us of continuous work to
            # reach 2.4GHz, and every engine (PE included) is stuck in
            # sequencer init until ~8.5us while the input DMAs land at
            # ~11us.  Dummy matmuls on a memset SBUF tile bridge PE-init
            # to data-ready so the clock ramp runs during the DMA wait
            # instead of during stage 1.  NWARM is sized to just cover
            # that ~2.5us bridge -- oversizing delays the real work (the
            # dummies serialize via PSUM WAW); the warm tile is never read.
            wu_sb = cpool.tile([P, 2, 256], FP8, tag="wu")
            nc.gpsimd.memset(wu_sb, 0)
            # the warm tile shares the stage-1 pool (same shape/tag): it
            # frees as soon as the last dummy retires (PE is serial), so
            # all four stage-1 m-tiles get their own buffer and never wait
            # on a PSUM->SBUF copy to drain
            wu_ps = ps1.tile([P, T], F32, tag="p1", name="wu_ps")
            for _ in range(11):
                nc.tensor.matmul(
                    wu_ps[:, :256],
                    wu_sb[:, :, 0:P],
                    wu_sb,
                    start=True,
                    stop=True,
                    perf_mode=mybir.MatmulPerfMode.DoubleRow,
                    skip_group_check=True,
                )
            # critical stage-1 operands first, one single-call DMA per
            # ring (each HWDGE ring is FIFO, and a DMA's completion sem
            # only fires once all 16 SDMA engines finish it, so whatever
            # is queued earlier completes earlier): gt on the SP ring
            # (sync), x then sc on the ACT ring (scalar).  W streams
            # behind as single chunks in stage-2 consumption order,
            # alternating rings, so chunk j0's sem fires ~right after
            # x/gt instead of at the end of the whole W burst.
            # x and gt as single whole-tile DMAs: the PE warmup bridges
            # until they land, and an unsplit stage 1 then runs gapless
            # (mid-stage stalls reset the PE p-state, costing more than
            # an earlier start saves).
            x_sb = xpool.tile([P, KP, 2, IN], FP8, tag="x")
            nc.scalar.dma_start(x_sb, x)
            gt_sb = cpool.tile([P, KP, 2, T], FP8, tag="gt")
            nc.sync.dma_start(gt_sb, gt)
            sc_sb = cpool.tile([P, 2], F32, tag="sc")
            nc.scalar.dma_start(sc_sb, sc)
            w_sb = cpool.tile([P, NCH, KP, 2, 512], FP8, tag="w")
            for j in range(NCH):
                eng = nc.sync if j % 2 == 0 else nc.scalar
                eng.dma_start(w_sb[:, j], w[:, j])

            # stage 1: xgT[i, t] = sum_s x_c[s, i] * GT[s, t]
            # GT[s, t] == 0 for t < s: s-tile kp only feeds t >= 256*kp.
            # ps1 bufs=4 so all four m-tiles run gapless on the PE; the
            # PSUM->SBUF scale copies split across VectorE (m0, m1 -> xg0,
            # gating the kp0 half of stage 2) and ScalarE (m2, m3 -> xg1).
            xg_sb = [
                xgpool.tile([P, 2, T], FP8, tag=f"xgp{kp}", name=f"xg{kp}")
                for kp in range(KP)
            ]
            for m in range(KI):
                p1 = ps1.tile([P, T], F32, tag="p1")
                for kp in range(KP):
                    t0 = kp * 2 * P
                    nc.tensor.matmul(
                        p1[:, t0:],
                        x_sb[:, kp, :, m * P : (m + 1) * P],
                        gt_sb[:, kp, :, t0:],
                        start=(kp == 0),
                        stop=(kp == KP - 1),
                        perf_mode=mybir.MatmulPerfMode.DoubleRow,
                        skip_group_check=True,
                    )
                # each copy is split in column halves across VectorE and
                # ScalarE so every xg gate closes ~0.3us after its matmul
                # instead of a full serial copy later (the m1/m3 copies
                # gate the two stage-2 contraction phases)
                dst = xg_sb[m // 2][:, m % 2, :]
                nc.vector.tensor_scalar(
                    dst[:, 0 : T // 2],
                    p1[:, 0 : T // 2],
                    sc_sb[:, 0:1],
                    None,
                    op0=mybir.AluOpType.mult,
                )
                nc.scalar.activation(
                    dst[:, T // 2 : T],
                    p1[:, T // 2 : T],
                    mybir.ActivationFunctionType.Copy,
                    scale=sc_sb[:, 0:1],
                )

            # stage 2: mem[t, n] = sum_i xgT[i, t] * W[i, n].  Per n-chunk
            # j, the kp0 partials for all four t-tiles run first (they only
            # need xg0), then kp1 closes them, so stage 2 starts as soon as
            # the first xg pair lands.  Consumption of the closed [128,512]
            # PSUM tiles alternates VectorE max-reduce / ScalarE Relu-accum
            # so neither engine back-pressures the PE.
            mx_sb = cpool.tile([P, NCH * KT + 1], F32, tag="mx")
            for j in range(NCH):
                p2s = []
                for mt in range(KT):
                    p2 = ps2.tile([P, 512], F32, tag="p2", name=f"p2_{j}_{mt}")
                    p2s.append(p2)
                    nc.tensor.matmul(
                        p2,
                        xg_sb[0][:, :, mt * P : (mt + 1) * P],
                        w_sb[:, j, 0],
                        start=True,
                        stop=False,
                        perf_mode=mybir.MatmulPerfMode.DoubleRow,
                        skip_group_check=True,
                    )
                for mt in range(KT):
                    p2 = p2s[mt]
                    nc.tensor.matmul(
                        p2,
                        xg_sb[1][:, :, mt * P : (mt + 1) * P],
                        w_sb[:, j, 1],
                        start=False,
                        stop=True,
                        perf_mode=mybir.MatmulPerfMode.DoubleRow,
                        skip_group_check=True,
                    )
                    idx = j * KT + mt
                    if idx == NCH * KT - 1:
                        # the very last tile sits on the critical tail:
                        # consume its halves on both engines in parallel
                        # (VectorE max -> col 31, ScalarE Relu -> col 32)
                        nc.vector.tensor_reduce(
                            mx_sb[:, idx : idx + 1],
                            p2[:, 0:384],
                            axis=mybir.AxisListType.X,
                            op=mybir.AluOpType.max,
                        )
                        junk = jpool.tile([P, 512], FP8, tag="junk")
                        nc.scalar.activation(
                            junk[:, 0:128],
                            p2[:, 384:512],
                            mybir.ActivationFunctionType.Relu,
                            bias=sc_sb[:, 1:2],
                            accum_out=mx_sb[:, idx + 1 : idx + 2],
                        )
                    elif IS_MAX[idx]:
                        nc.vector.tensor_reduce(
                            mx_sb[:, idx : idx + 1],
                            p2,
                            axis=mybir.AxisListType.X,
                            op=mybir.AluOpType.max,
                        )
                    else:
                        junk = jpool.tile([P, 512], FP8, tag="junk")
                        nc.scalar.activation(
                            junk,
                            p2,
                            mybir.ActivationFunctionType.Relu,
                            bias=sc_sb[:, 1:2],
                            accum_out=mx_sb[:, idx : idx + 1],
                        )
                if j == NCH - 3:
                    # first 3/4 of the results ship while the last two
                    # n-chunks still compute, hiding the DMA latency
                    nc.sync.dma_start(
                        mx[:, : (j + 1) * KT], mx_sb[:, : (j + 1) * KT]
                    )
            nc.sync.dma_start(
                mx[:, (NCH - 2) * KT :], mx_sb[:, (NCH - 2) * KT :]
            )
    nc.compile()
    return nc


def _pow2_scale(target_max: float, value_max: float) -> float:
    """Largest power of two s with value_max * s <= target_max."""
    if value_max <= 0 or not np.isfinite(value_max):
        return 1.0
    return 2.0 ** math.floor(math.log2(target_max / value_max))


def _run_spmd_with_retry(nc, in_maps, trace=False, tries=4):
    """run_bass_kernel_spmd with retry: execution occasionally dies with a
    transient NRT error (device left wedged by a previous process).  A
    plain retry usually fails in-process, so later attempts reset the jax
    backend to get a fresh PJRT client."""
    import time as _time

    last = None
    for attempt in range(tries):
        try:
            return bass_utils.run_bass_kernel_spmd(
                nc, in_maps, core_ids=list(range(NCORES)), trace=trace
            )
        except Exception as e:  # noqa: BLE001
            last = e
            _time.sleep(2.0)
            try:
                import jax

                jax.clear_caches()
                jax.extend.backend.clear_backends()
            except Exception:  # noqa: BLE001
                pass
    raise last


def _run_device(x_bm, W_in, gt_np, th_scaled, sx, sw, sxx, sgt, trace=False):
    """Run the SPMD kernel; returns (mx [NCORES, P, NCH*KT] f32, res).

    mx even columns: VectorE tile max of mem*sx*sw.
    mx odd columns:  ScalarE tile sum of Relu(mem*sx*sw - th_scaled/2).
    """
    if "v2" not in _compiled:
        _compiled["v2"] = _build_device()
    nc = _compiled["v2"]
    # fp8 stage-1 operands with power-of-two scales sxx (x) and sgt (gt)
    x_f8 = (x_bm.astype(np.float64) * sxx).astype(np.float32).astype(NPFP8)
    gt_f8 = (gt_np.astype(np.float64) * sgt).astype(np.float32).astype(NPFP8)
    # gt[p, kp, i2, t] = GT[(2kp+i2)*128+p, t] * sgt
    gt_pack = np.ascontiguousarray(
        gt_f8.reshape(KP, 2, P, T).transpose(2, 0, 1, 3)
    )
    # x[b][p, kp, i2, i] = x_b[(2kp+i2)*128+p, i] * sxx
    x_pack_all = np.ascontiguousarray(
        x_f8.reshape(B, KP, 2, P, IN).transpose(0, 3, 1, 2, 4)
    )
    w_fp8 = (W_in.astype(np.float64) * sw).astype(np.float32).astype(NPFP8)
    # w[p, j, kp, i2, n] = W_in[(2kp+i2)*128+p, j*512+n] * sw
    w_pack = np.ascontiguousarray(
        w_fp8.reshape(KP, 2, P, NCH, 512).transpose(2, 3, 0, 1, 4)
    )
    sc_arr = np.empty((P, 2), dtype=np.float32)
    sc_arr[:, 0] = sx / (sxx * sgt)
    sc_arr[:, 1] = -0.5 * th_scaled
    in_maps = [
        {
            "x": np.ascontiguousarray(x_pack_all[c]),
            "w": w_pack,
            "gt": gt_pack,
            "sc": sc_arr,
        }
        for c in range(NCORES)
    ]
    res = _run_spmd_with_retry(nc, in_maps, trace=trace)
    global LAST_RES
    LAST_RES = res
    mx = np.stack(
        [res.results[c]["mx"].astype(np.float32) for c in range(NCORES)]
    )
    return mx, res


def _fallback(input_signal, weights, tau_mem, tau_syn, threshold):
    """Exact sequential port of the reference (numpy float32)."""
    x = np.asarray(input_signal, dtype=np.float32)
    w = np.asarray(weights, dtype=np.float32)
    W_in, W_rec = w[:IN], w[IN:]
    Tt, Bb, Nn = x.shape
    ff = np.einsum("tbi,in->tbn", x[:, :, :IN], W_in).astype(np.float32)
    syn = np.zeros((Bb, Nn), np.float32)
    mem = np.zeros((Bb, Nn), np.float32)
    fb = np.zeros((Bb, Nn), np.float32)
    out = np.zeros((Tt, Bb, Nn), np.float32)
    for t in range(Tt):
        cur = ff[t] + fb
        syn = syn + (-syn / tau_syn + cur) * np.float32(DT)
        mem = mem + (-mem / tau_mem + syn) * np.float32(DT)
        spikes = (mem >= threshold).astype(np.float32)
        mem = mem * (1.0 - spikes)
        rec = spikes[:, IN:] @ W_rec
        rec[:, :IN] = 0.0
        fb = rec
        out[t] = spikes
    return out


def kernel(input_signal, weights, tau_mem, tau_syn, threshold, _trace=False):
    input_signal = np.asarray(input_signal)
    weights = np.asarray(weights)
    tau_mem = np.asarray(tau_mem)
    tau_syn = np.asarray(tau_syn)
    threshold = np.asarray(threshold)

    ok_shape = (
        input_signal.shape == (T, B, N)
        and weights.shape == (N, N)
        and np.all(tau_mem == tau_mem.flat[0])
        and np.all(tau_syn == tau_syn.flat[0])
        and np.all(np.isfinite(input_signal))
        and np.all(np.isfinite(weights[:IN]))
        and np.all(np.isfinite(threshold))
    )
    if not ok_shape:
        return _fallback(input_signal, weights, tau_mem, tau_syn, threshold)

    alpha = 1.0 - DT / float(tau_syn.flat[0])
    beta = 1.0 - DT / float(tau_mem.flat[0])
    if not (0.0 <= alpha < 1.0 and 0.0 <= beta < 1.0):
        # numerically unstable / nonstandard regime: be safe
        return _fallback(input_signal, weights, tau_mem, tau_syn, threshold)

    gt_np = _build_gt(alpha, beta)

    # --- rigorous sub-threshold bound (exact arithmetic) -----------------
    # mem = xg @ W with
    # |xg[i,t]| <= max_col||x_col||_2 * max_col||gt_col||_2
    # |mem[t,n]| <= ||xg[:,t]||_2 * ||W[:,n]||_2
    #            <= sum_d g(d)DT^2 * max_row||x_row||_2 * max_col||W_col||_2
    x_in = input_signal[:, :, :IN].astype(np.float64)
    W_in64 = weights[:IN].astype(np.float64)
    max_row = float(np.sqrt((x_in * x_in).sum(axis=2).max()))
    max_wcol = float(np.sqrt((W_in64 * W_in64).sum(axis=0).max()))
    gsum = float(_filter_taps(alpha, beta).sum())
    mem_bound = gsum * max_row * max_wcol

    # fp8 scale factors from data maxima / bounds (powers of two, exact)
    xcol_max = float(np.sqrt((x_in * x_in).sum(axis=0).max()))
    gtcol_max = float(np.sqrt((gt_np.astype(np.float64) ** 2).sum(axis=0).max()))
    xg_bound = xcol_max * gtcol_max
    w_max = float(np.abs(W_in64).max())
    x_max = float(np.abs(x_in).max())
    gt_max = float(np.abs(gt_np).max())
    sx = _pow2_scale(224.0, xg_bound)
    sw = _pow2_scale(224.0, w_max)
    sxx = _pow2_scale(224.0, x_max)
    sgt = _pow2_scale(224.0, gt_max)

    # --- mixed-precision error allowance (conservative, absolute) -------
    # All operands are fp8-e4m3: per-operand rounding <= 2^-4 relative
    # plus a subnormal-flush floor eps = 2^-9/scale; products accumulate
    # exactly in fp32 PSUM.  Stage-1 error |dxg| propagates through the
    # (exactly bounded) stage-2 weights.
    eps_xx = 2.0**-9 / sxx
    eps_gt = 2.0**-9 / sgt
    xg_err = (
        0.14 * xg_bound
        + T * (eps_xx * gt_max + eps_gt * x_max + eps_xx * eps_gt)
    )
    eps_w = 2.0**-9 / sw
    err = (
        0.15 * mem_bound
        + IN * ((2.0**-9 / sx) * w_max + eps_w * xg_bound + eps_w * xg_err)
        + IN * xg_err * w_max * 1.15
    )
    safe = (mem_bound + err) < float(threshold.min()) - MARGIN
    if not safe:
        return _fallback(input_signal, weights, tau_mem, tau_syn, threshold)

    # batch-major rows: row (b*T + t) = input_signal[t, b, :IN]
    x_bm = np.ascontiguousarray(
        input_signal[:, :, :IN].transpose(1, 0, 2).reshape(B * T, IN)
    ).astype(np.float32, copy=False)
    W_in = np.ascontiguousarray(weights[:IN]).astype(np.float32, copy=False)

    th_scaled = float(threshold.min()) * sx * sw
    try:
        mx, _ = _run_device(
            x_bm, W_in, gt_np, th_scaled, sx, sw, sxx, sgt, trace=_trace
        )
    except Exception:  # device unusable: still return a correct result
        return _fallback(input_signal, weights, tau_mem, tau_syn, threshold)
    # even columns: exact fp32 tile maxes of mem*sx*sw -- require a 2x
    # margin to the scaled threshold; odd columns: sums of
    # Relu(mem*sx*sw - th/2) -- any nonzero means some value reached
    # th/2.  Both are far stronger than the is_ge the reference applies;
    # anything close is recomputed exactly on the host.
    if not np.isfinite(mx).all():
        return _fallback(input_signal, weights, tau_mem, tau_syn, threshold)
    is_max = np.array(IS_MAX)
    mx_max = float(mx[:, :, is_max].max())
    relu_sum = float(np.abs(mx[:, :, ~is_max]).max())
    if mx_max >= 0.5 * th_scaled or relu_sum > 0.0:
        return _fallback(input_signal, weights, tau_mem, tau_syn, threshold)
    return np.zeros((T, B, N), dtype=np.float32)



# revision 10
# speedup vs baseline: 1.7407x; 1.7407x over previous
"""Trainium2 Bass kernel for nn_EvolvableSNN (T=512, B=8, N=4096, LIF SNN).

Strategy
--------
The LIF dynamics with these parameters are sub-threshold: the membrane
potential equilibrium is ~tau_mem*tau_syn*cur ~= 1e-4 * cur, four orders of
magnitude below threshold=1.0, so no neuron ever spikes and the recurrent
feedback term is identically zero.  With zero feedback the scan is a LINEAR
time-invariant filter of the feedforward drive:

    ff    = input[:, :, :512] @ W_in                      # [T, B, N]
    mem_t = DT^2 * sum_{s<=t} g(t-s) * ff_s               # per (b, n)
    g(d)  = (b^(d+1) - a^(d+1)) / (b - a),  a = 1-DT/tau_syn, b = 1-DT/tau_mem
    spikes_t = (mem_t >= threshold)

so mem = (x @_time GT) @ W_in, fully parallel across (batch, neuron).
Validity is guarded by a rigorous norm bound computed on the host:

    max|mem| <= DT^2 * sum_d g(d) * max_row||x_row||_2 * max_col||W_col||_2

(~2e-3 for the target inputs, vs threshold 1.0).  If the bound (inflated by
the mixed-precision error allowance) does not clear min(threshold) by a wide
margin -- or the device-computed certificate comes anywhere near threshold --
we fall back to an exact sequential numpy port of the reference.  The first
spike of the no-feedback system coincides with the first spike of the true
system, so "no spikes under linearization" exactly implies correctness.

Device kernel (per core, batch-parallel: core c owns batch c, full N):
  stage 1: xgT[i, t] = sum_s x_c[s, i] * GT[s, t]   (fp8 DoubleRow matmuls,
           GT upper-triangular so the moving range is trimmed); the
           PSUM->SBUF copies apply |.|*cscale and cast to fp8.
  stage 2: C[t] = sum_i |xg[i, t]| * Wmax[i]        (2 fp8 DoubleRow
           matmuls with the [128,2,1] Wmax column as the stationary
           operand -> a [1, 512] PSUM row)
  where Wmax[i] = max_n |W_in[i, n]| is computed on the host and rounded
  UP in fp8, so C[t] is a sound upper bound (up to the host-accounted
  stage-1 fp8 error) on max_n |mem[t, n]| * sx * sw:

    |mem[t,n]| = |sum_i xg[t,i] W[i,n]| <= sum_i |xg[t,i]| Wmax[i]

  The host checks max_t C < 0.5*threshold*sx*sw - slack (slack covers all
  fp8 rounding, exactly bounded), then emits the all-zero spike tensor;
  anything unexpected falls back to the exact numpy path.  The only device
  output is the [1, 512] C row (2 KB) -- no spike map is materialized.

Numerics: both matmul stages run as fp8-e4m3 DoubleRow (2x PE throughput)
with power-of-two scale factors (sxx on x, sgt on GT, sx/(sxx*sgt) applied
by the PSUM->SBUF abs-copy, sw folded into Wmax on the host); accumulation
is fp32 PSUM throughout, and C is an exact fp32 contraction of nonnegative
fp8 values (no cancellation).
"""

import math

import numpy as np
import ml_dtypes

import concourse.bass as bass
import concourse.mybir as mybir
import concourse.tile as tile
from concourse import bacc, bass_utils

# Problem constants (hardcoded per harness contract).
T, B, N = 512, 8, 4096
IN = 512          # INPUT_SIZE
DT = 0.001
P = 128           # SBUF partitions
NCORES = 8

KI = IN // P      # contraction tiles over input dim (4)
KP = KI // 2      # DoubleRow contraction pair-tiles (2)
F32 = mybir.dt.float32
FP8 = mybir.dt.float8e4
NPFP8 = ml_dtypes.float8_e4m3

MARGIN = 0.1               # abs margin to min(threshold) for the fast path
NWARM = 6                  # PE p-state warmup dummy matmuls

_compiled = {}             # cached compiled Bass modules
LAST_RES = None            # last device results (for external profiling)


def _filter_taps(alpha: float, beta: float) -> np.ndarray:
    """g(d) * DT^2 for d = 0..T-1 (float64)."""
    d = np.arange(T, dtype=np.float64)
    if abs(beta - alpha) > 1e-12:
        g = (beta ** (d + 1) - alpha ** (d + 1)) / (beta - alpha)
    else:
        g = (d + 1) * alpha**d
    return g * DT * DT


def _build_gt(alpha: float, beta: float) -> np.ndarray:
    """GT[s, t] = DT^2 * g(t - s) for s <= t else 0 (upper-triangular)."""
    g = _filter_taps(alpha, beta)
    s = np.arange(T)
    diff = s[None, :] - s[:, None]  # diff[s, t] = t - s
    gt = np.where(diff >= 0, g[np.clip(diff, 0, T - 1)], 0.0)
    return gt.astype(np.float32)


def _fp8_roundup(v: np.ndarray) -> np.ndarray:
    """Smallest fp8-e4m3 >= v (v float64, 0 <= v <= 224)."""
    r = v.astype(np.float32).astype(NPFP8)
    lt = r.astype(np.float64) < v
    bits = r.view(np.uint8)
    bits = np.where(lt, bits + 1, bits).astype(np.uint8)
    return bits.view(NPFP8)


def _choose_scales(xg_bound: float, x_max: float, gt_max: float):
    """Power-of-two sxx, sgt with sxx*sgt == sx == pow2(224/xg_bound).

    The stage-1 PSUM is then xg*sx directly, so the PSUM->SBUF abs copy
    needs NO scale (pure |.|, which both VectorE tensor_reduce and
    ScalarE Abs support).  The split is balanced to minimize the fp8
    subnormal-flush floors T*(eps_xx*gt_max + eps_gt*x_max), clamped so
    neither operand overflows fp8.
    """
    sx = _pow2_scale(224.0, xg_bound)
    sxx_cap = _pow2_scale(224.0, x_max)
    sgt_cap = _pow2_scale(224.0, gt_max)
    if sx > sxx_cap * sgt_cap:
        return None  # cannot represent: caller falls back
    a_opt = 0.5 * (math.log2(sx) + math.log2(max(gt_max, 1e-300) / max(x_max, 1e-300)))
    sxx = 2.0 ** round(a_opt)
    sxx = min(sxx, sxx_cap)
    sgt = sx / sxx
    if sgt > sgt_cap:
        sgt = sgt_cap
        sxx = sx / sgt
    if sxx > sxx_cap:
        return None
    return sx, sxx, sgt


def _build_device():
    """Compile the per-core Tile kernel; returns the Bass module.

    Input layouts are pre-packed on the host so every DMA is one large
    fully-contiguous transfer:
      x  [P, KP, 2, IN]   fp8, x[p, kp, i2, i] = x_c[(2kp+i2)*128+p, i] * sxx
      gt [P, KP, 2, T]    fp8, gt[p, kp, i2, t] = GT[(2kp+i2)*128+p, t] * sgt
      wm [P, KP, 2, 16]   fp8, wm[p, kp, i2, 0] = roundup(Wmax[(2kp+i2)*128+p] * sw),
                          cols 1..15 zero (pad: dual-fp8 LDWEIGHTS needs a
                          16B-aligned even step on the i2 pair axis)
    Output:
      mx [1, T]           f32, C[t] = sum_i |xg8[i, t]| * wm8[i]

    sxx*sgt == sx, so stage-1 PSUM is xg*sx and the abs copies are
    scale-free.  Only the two HWDGE rings are used: critical stage-1
    operands first (gt on sync, x on scalar), the tiny wm pad behind gt.
    """
    nc = bacc.Bacc(
        "TRN2", target_bir_lowering=False, debug=False, num_devices=NCORES
    )
    x = nc.dram_tensor("x", [P, KP, 2, IN], FP8, kind="ExternalInput").ap()
    gt = nc.dram_tensor("gt", [P, KP, 2, T], FP8, kind="ExternalInput").ap()
    wm = nc.dram_tensor("wm", [P, KP, 2, 16], FP8, kind="ExternalInput").ap()
    mx = nc.dram_tensor("mx", [1, T], F32, kind="ExternalOutput").ap()

    with tile.TileContext(nc) as tc:
        with (
            tc.tile_pool(name="const", bufs=1) as cpool,
            tc.tile_pool(name="xin", bufs=1) as xpool,
            tc.tile_pool(name="xg", bufs=1) as xgpool,
            tc.tile_pool(name="ps1", bufs=4, space="PSUM") as ps1,
            tc.tile_pool(name="ps2", bufs=1, space="PSUM") as ps2,
        ):
            # PE p-state warmup: every engine is stuck in sequencer init
            # until ~6.5us and the input DMAs land ~2us later.  Dummy
            # matmuls on a memset SBUF tile bridge PE-init to data-ready
            # so the clock ramp runs during the DMA wait instead of
            # during stage 1.  The warm tile shares the stage-1 pool
            # (same shape/tag): it frees as soon as the last dummy
            # retires (PE is serial).
            wu_sb = cpool.tile([P, 2, 256], FP8, tag="wu")
            nc.gpsimd.memset(wu_sb, 0)
            wu_ps = ps1.tile([P, T], F32, tag="p1", name="wu_ps")
            for _ in range(NWARM):
                nc.tensor.matmul(
                    wu_ps[:, :256],
                    wu_sb[:, :, 0:P],
                    wu_sb,
                    start=True,
                    stop=True,
                    perf_mode=mybir.MatmulPerfMode.DoubleRow,
                    skip_group_check=True,
                )
            # critical stage-1 operands first, one single-call DMA per
            # ring: gt on the SP ring (sync), x on the ACT ring (scalar).
            # The 512-byte wm column queues behind gt.
            gt_sb = cpool.tile([P, KP, 2, T], FP8, tag="gt")
            nc.sync.dma_start(gt_sb, gt)
            x_sb = xpool.tile([P, KP, 2, IN], FP8, tag="x")
            nc.scalar.dma_start(x_sb, x)
            wm_sb = cpool.tile([P, KP, 2, 16], FP8, tag="wm")
            nc.sync.dma_start(wm_sb, wm)

            # stage 1: xgT[i, t] = sum_s x_c[s, i] * GT[s, t]
            # GT[s, t] == 0 for t < s: s-tile kp only feeds t >= 256*kp.
            # ps1 bufs=4 so all four m-tiles run gapless on the PE; the
            # PSUM->SBUF |.|*cscale copies split in column halves across
            # VectorE (abs_max against 0) and ScalarE (Abs activation) so
            # each xg gate closes ~0.35us after its matmul.
            xg_sb = [
                xgpool.tile([P, 2, T], FP8, tag=f"xgp{kp}", name=f"xg{kp}")
                for kp in range(KP)
            ]
            for m in range(KI):
                p1 = ps1.tile([P, T], F32, tag="p1")
                for kp in range(KP):
                    t0 = kp * 2 * P
                    nc.tensor.matmul(
                        p1[:, t0:],
                        x_sb[:, kp, :, m * P : (m + 1) * P],
                        gt_sb[:, kp, :, t0:],
                        start=(kp == 0),
                        stop=(kp == KP - 1),
                        perf_mode=mybir.MatmulPerfMode.DoubleRow,
                        skip_group_check=True,
                    )
                dst = xg_sb[m // 2][:, m % 2, :]
                nc.vector.tensor_reduce(
                    dst[:, 0 : T // 2],
                    p1[:, 0 : T // 2].unsqueeze(-1),
                    axis=mybir.AxisListType.X,
                    op=mybir.AluOpType.max,
                    apply_absolute_value=True,
                )
                nc.scalar.activation(
                    dst[:, T // 2 : T],
                    p1[:, T // 2 : T],
                    mybir.ActivationFunctionType.Abs,
                    scale=1.0,
                )

            # stage 2: C[t] = sum_i xg8[i, t] * wm8[i] -- the Wmax pad
            # is the stationary operand (16 output partitions, rows 1..15
            # zero), xg is the moving operand, so each kp half is ONE
            # 512-wide matmul.
            p2 = ps2.tile([16, T], F32, tag="p2")
            for kp in range(KP):
                nc.tensor.matmul(
                    p2,
                    wm_sb[:, kp],
                    xg_sb[kp],
                    start=(kp == 0),
                    stop=(kp == KP - 1),
                    perf_mode=mybir.MatmulPerfMode.DoubleRow,
                    skip_group_check=True,
                )
            # PSUM -> SBUF -> HBM; halves on VectorE/ScalarE in parallel
            mx_sb = cpool.tile([1, T], F32, tag="mx")
            nc.vector.tensor_scalar(
                mx_sb[:, 0 : T // 2],
                p2[0:1, 0 : T // 2],
                1.0,
                None,
                op0=mybir.AluOpType.mult,
            )
            nc.scalar.activation(
                mx_sb[:, T // 2 : T],
                p2[0:1, T // 2 : T],
                mybir.ActivationFunctionType.Copy,
                scale=1.0,
            )
            nc.sync.dma_start(mx, mx_sb)
    nc.compile()
    return nc


def _pow2_scale(target_max: float, value_max: float) -> float:
    """Largest power of two s with value_max * s <= target_max."""
    if value_max <= 0 or not np.isfinite(value_max):
        return 1.0
    return 2.0 ** math.floor(math.log2(target_max / value_max))


def _run_spmd_with_retry(nc, in_maps, trace=False, tries=4):
    """run_bass_kernel_spmd with retry: execution occasionally dies with a
    transient NRT error (device left wedged by a previous process).  A
    plain retry usually fails in-process, so later attempts reset the jax
    backend to get a fresh PJRT client."""
    import time as _time

    last = None
    for attempt in range(tries):
        try:
            return bass_utils.run_bass_kernel_spmd(
                nc, in_maps, core_ids=list(range(NCORES)), trace=trace
            )
        except Exception as e:  # noqa: BLE001
            last = e
            _time.sleep(2.0)
            try:
                import jax

                jax.clear_caches()
                jax.extend.backend.clear_backends()
            except Exception:  # noqa: BLE001
                pass
    raise last


def _run_device(x_bm, wmax8, gt_np, sxx, sgt, trace=False):
    """Run the SPMD kernel; returns (mx [NCORES, 1, T] f32, res).

    mx[c, 0, t] = sum_i xg8[i, t] * wm8[i] for batch c (nonneg, fp32).
    """
    if "v4" not in _compiled:
        _compiled["v4"] = _build_device()
    nc = _compiled["v4"]
    # fp8 stage-1 operands with power-of-two scales sxx (x) and sgt (gt)
    x_f8 = (x_bm.astype(np.float64) * sxx).astype(np.float32).astype(NPFP8)
    gt_f8 = (gt_np.astype(np.float64) * sgt).astype(np.float32).astype(NPFP8)
    # gt[p, kp, i2, t] = GT[(2kp+i2)*128+p, t] * sgt
    gt_pack = np.ascontiguousarray(
        gt_f8.reshape(KP, 2, P, T).transpose(2, 0, 1, 3)
    )
    # x[b][p, kp, i2, i] = x_b[(2kp+i2)*128+p, i] * sxx
    x_pack_all = np.ascontiguousarray(
        x_f8.reshape(B, KP, 2, P, IN).transpose(0, 3, 1, 2, 4)
    )
    # wm[p, kp, i2, 0] = wmax8[(2kp+i2)*128+p]  (pre-rounded-up fp8),
    # cols 1..15 zero padding
    wm_pack = np.zeros((P, KP, 2, 16), dtype=NPFP8)
    wm_pack[:, :, :, 0] = wmax8.reshape(KP, 2, P).transpose(2, 0, 1)
    in_maps = [
        {
            "x": np.ascontiguousarray(x_pack_all[c]),
            "gt": gt_pack,
            "wm": wm_pack,
        }
        for c in range(NCORES)
    ]
    res = _run_spmd_with_retry(nc, in_maps, trace=trace)
    global LAST_RES
    LAST_RES = res
    mx = np.stack(
        [res.results[c]["mx"].astype(np.float32) for c in range(NCORES)]
    )
    return mx, res


def _fallback(input_signal, weights, tau_mem, tau_syn, threshold):
    """Exact sequential port of the reference (numpy float32)."""
    x = np.asarray(input_signal, dtype=np.float32)
    w = np.asarray(weights, dtype=np.float32)
    W_in, W_rec = w[:IN], w[IN:]
    Tt, Bb, Nn = x.shape
    ff = np.einsum("tbi,in->tbn", x[:, :, :IN], W_in).astype(np.float32)
    syn = np.zeros((Bb, Nn), np.float32)
    mem = np.zeros((Bb, Nn), np.float32)
    fb = np.zeros((Bb, Nn), np.float32)
    out = np.zeros((Tt, Bb, Nn), np.float32)
    for t in range(Tt):
        cur = ff[t] + fb
        syn = syn + (-syn / tau_syn + cur) * np.float32(DT)
        mem = mem + (-mem / tau_mem + syn) * np.float32(DT)
        spikes = (mem >= threshold).astype(np.float32)
        mem = mem * (1.0 - spikes)
        rec = spikes[:, IN:] @ W_rec
        rec[:, :IN] = 0.0
        fb = rec
        out[t] = spikes
    return out


def kernel(input_signal, weights, tau_mem, tau_syn, threshold, _trace=False):
    input_signal = np.asarray(input_signal)
    weights = np.asarray(weights)
    tau_mem = np.asarray(tau_mem)
    tau_syn = np.asarray(tau_syn)
    threshold = np.asarray(threshold)

    ok_shape = (
        input_signal.shape == (T, B, N)
        and weights.shape == (N, N)
        and np.all(tau_mem == tau_mem.flat[0])
        and np.all(tau_syn == tau_syn.flat[0])
        and np.all(np.isfinite(input_signal))
        and np.all(np.isfinite(weights[:IN]))
        and np.all(np.isfinite(threshold))
    )
    if not ok_shape:
        return _fallback(input_signal, weights, tau_mem, tau_syn, threshold)

    alpha = 1.0 - DT / float(tau_syn.flat[0])
    beta = 1.0 - DT / float(tau_mem.flat[0])
    if not (0.0 <= alpha < 1.0 and 0.0 <= beta < 1.0):
        # numerically unstable / nonstandard regime: be safe
        return _fallback(input_signal, weights, tau_mem, tau_syn, threshold)

    gt_np = _build_gt(alpha, beta)

    # --- rigorous sub-threshold bound (exact arithmetic) -----------------
    # mem = xg @ W with
    # |xg[i,t]| <= max_col||x_col||_2 * max_col||gt_col||_2
    # |mem[t,n]| <= ||xg[:,t]||_2 * ||W[:,n]||_2
    #            <= sum_d g(d)DT^2 * max_row||x_row||_2 * max_col||W_col||_2
    x_in = input_signal[:, :, :IN].astype(np.float64)
    W_in64 = weights[:IN].astype(np.float64)
    max_row = float(np.sqrt((x_in * x_in).sum(axis=2).max()))
    max_wcol = float(np.sqrt((W_in64 * W_in64).sum(axis=0).max()))
    gsum = float(_filter_taps(alpha, beta).sum())
    mem_bound = gsum * max_row * max_wcol

    # fp8 scale factors from data maxima / bounds (powers of two, exact)
    xcol_max = float(np.sqrt((x_in * x_in).sum(axis=0).max()))
    gtcol_max = float(np.sqrt((gt_np.astype(np.float64) ** 2).sum(axis=0).max()))
    xg_bound = xcol_max * gtcol_max
    wmax = np.abs(W_in64).max(axis=1)       # Wmax[i] = max_n |W_in[i, n]|
    w_max = float(wmax.max())
    x_max = float(np.abs(x_in).max())
    gt_max = float(np.abs(gt_np).max())
    scales = _choose_scales(xg_bound, x_max, gt_max)
    if scales is None:
        return _fallback(input_signal, weights, tau_mem, tau_syn, threshold)
    sx, sxx, sgt = scales
    sw = _pow2_scale(224.0, w_max)

    # --- mixed-precision error allowance (conservative, absolute) -------
    # All operands are fp8-e4m3: per-operand rounding <= 2^-4 relative
    # plus a subnormal-flush floor eps = 2^-9/scale; products accumulate
    # in fp32 PSUM.  xg_err bounds |xg8/sx - xg_true| elementwise (the
    # 0.21 covers the x/gt input rounding through the stage-1 contraction
    # plus the |.| copy's own fp8 rounding; the T*(...) term the
    # subnormal floors).
    eps_xx = 2.0**-9 / sxx
    eps_gt = 2.0**-9 / sgt
    xg_err = (
        0.21 * xg_bound
        + 1.1 * T * (eps_xx * gt_max + eps_gt * x_max + eps_xx * eps_gt)
        + 2.0**-8 / sx
    )
    # host-side check that the linearized mem stays far below threshold
    eps_w = 2.0**-9 / sw
    err = (
        0.15 * mem_bound
        + IN * (xg_err * (w_max + eps_w) + (xg_bound + xg_err) * eps_w) * 1.15
    )
    safe = (mem_bound + err) < float(threshold.min()) - MARGIN
    if not safe:
        return _fallback(input_signal, weights, tau_mem, tau_syn, threshold)

    # batch-major rows: row (b*T + t) = input_signal[t, b, :IN]
    x_bm = np.ascontiguousarray(
        input_signal[:, :, :IN].transpose(1, 0, 2).reshape(B * T, IN)
    ).astype(np.float32, copy=False)

    # Wmax column, scaled and rounded UP in fp8 so the device C is a
    # sound upper bound on sum_i |xg8| * Wmax * sw
    wmax8 = _fp8_roundup(wmax * sw)

    try:
        mx, _ = _run_device(x_bm, wmax8, gt_np, sxx, sgt, trace=_trace)
    except Exception:  # device unusable: still return a correct result
        return _fallback(input_signal, weights, tau_mem, tau_syn, threshold)
    # Device certificate: for every (core, t),
    #   max_n |mem[t,n]| * sx * sw <= C[t] * (1+3e-4) + slack
    # with slack = sx * xg_err * sum_i wm8[i] covering the stage-1 fp8
    # error against the exact xg, and (1+3e-4) the fp32 PSUM accumulation
    # rounding of the 512-term nonneg dot product.
    if not np.isfinite(mx).all():
        return _fallback(input_signal, weights, tau_mem, tau_syn, threshold)
    s_w8 = float(wmax8.astype(np.float64).sum())
    slack = sx * xg_err * s_w8 + 2.0**-8 * s_w8
    c_max = float(mx.max())
    thr_scaled = 0.5 * float(threshold.min()) * sx * sw
    if c_max * 1.0003 + slack >= thr_scaled:
        return _fallback(input_signal, weights, tau_mem, tau_syn, threshold)
    return np.zeros((T, B, N), dtype=np.float32)
